# revision 25
# baseline (speedup 1.0000x reference)
"""Trainium2 Bass kernel: MeanHinAggregator (GNN message passing).

Reference computation (per batch-head element bh):
    z_r  = mean_n(x_neigh_r[bh, n, :]) @ w_neigh_r          (r = 0, 1)
    out  = relu(concat(x_self[bh] @ w_self, (z0 + z1) / 2) + b)

Strategy (pure data parallel over 8 NeuronCores, batch axis sharded):
  * Per core: B_shard=128, H=10 -> 1280 rows, processed in 10 groups of 128.
  * The kernel is memory-bound (44 MB/core fp32; all 8 cores share one
    NeuronDevice's HBM, so the real constraint is the ~2.8 TB/s device
    aggregate).  All streamed tensors are downcast to fp16 on the host
    (rel-err gate is 2e-2; fp16 keeps us at ~4e-4): halves DMA bytes,
    doubles DVE fold throughput (2x_1p mode for 2-byte packed dtypes),
    and makes PE matmuls single-pass (1 cycle/row vs 4 for fp32).
  * Host packs [xn0 | x_self | xn1] row-wise into one [BH, 65*F] tensor;
    each group is one [128, 65, 128] fp16 tile loaded by TWO DMAs, one per
    HWDGE ring (SP gets xn0+xs, ACT gets xn1) -> balanced queues, one DMA
    + one tile instance fewer per group than a separate x_self stream, and
    each partition line is a contiguous >=8 KiB descriptor (full DMA rate).
  * xpool bufs=3 doubles as HBM pacing: a group's loads are gated on the
    group-3-back compute, capping per-core run-ahead so no core hogs the
    oversubscribed device HBM (bufs=2 serializes DMA with compute: +25 us;
    bufs=5 lets fast cores starve stragglers and raises the max).
  * The mean-over-neighbours reduction: four in-place strided adds on the
    Vector engine per relation fold its 32 slices to 2 (fp16 2x_1p,
    ~0.52 ns/elem); folding relation 0 only needs the SP-ring DMA, and its
    transposes overlap relation 1's fold.  NOTE: do NOT offload fold work
    to GPSIMD — DVE and GPSIMD share SBUF ports, so running them
    concurrently halves both (measured: 90us -> 100us).
  * Two accumulating transposing matmuls per relation (lhsT = data slice,
    rhs = identity -> out[f, bh] = data[bh, f], PSUM accumulation sums the
    last 2 slices) put the operands in the [f, bh] layout the projection
    needs as lhsT (the PE contracts over the partition axis).
  * Projection: out[bh, d] = sumT.T @ w.  The 1/(N*NR) mean scaling is
    folded into host-prescaled fp16 copies of w_neigh_*.  Bias is added
    with a K=1 matmul (lhsT = ones row, rhs = bias row) into PSUM.
    PSUM -> SBUF copy and final ReLU run on the Scalar engine (keeps DVE
    fold-only).  ReLU emits fp16; the host upcasts to fp32 (halves store
    bytes).  Output stores ride the ACT ring (SP's packed load is 3%
    bigger, so this balances the rings).

Measured on HW: 137.7 us (fp32 baseline) -> ~82-88 us best-of-10
(run-to-run device drift is +-5 us; all 8 cores share one NeuronDevice's
HBM and the kernel runs at the device-aggregate memory roofline).
"""

import numpy as np

import concourse.bacc as bacc
import concourse.bass as bass
import concourse.tile as tile
from concourse import bass_utils, mybir
from concourse._compat import with_exitstack

B, H, N, F = 1024, 10, 32, 128
HALF = 128
D = 2 * HALF
NR = 2
NCORES = 8
BSH = B // NCORES        # 128 batch rows per core
BH = BSH * H             # 1280 (bh rows per core)
GROUP = 128              # bh rows per group
NF = N * F               # 4096 (one relation's row width)
ROW = 2 * NF + F         # 8320 packed row: [xn0 | xs | xn1]
XS_J = N                 # j-index of the x_self slice in the packed tile
R1_J = N + 1             # j-index where relation 1's slices start
F32 = mybir.dt.float32
F16 = mybir.dt.float16


@with_exitstack
def _tile_kernel(ctx, tc, outs, ins, ngroups):
    nc = tc.nc
    xn, cmat, bones = ins
    (out_d,) = outs

    const = ctx.enter_context(tc.tile_pool(name="const", bufs=1))
    xpool = ctx.enter_context(tc.tile_pool(name="xp", bufs=3))
    spool = ctx.enter_context(tc.tile_pool(name="sp", bufs=4))
    ppool = ctx.enter_context(tc.tile_pool(name="ps", bufs=3, space="PSUM"))

    def issue_loads(g):
        """One packed tile per group, split across both HWDGE rings:
        SP carries xn0+xs (columns 0:4224), ACT carries xn1."""
        r = slice(g * GROUP, (g + 1) * GROUP)
        t = xpool.tile([128, 2 * N + 1, F], F16, tag="t")
        nc.sync.dma_start(t[:, 0:R1_J, :], xn[r, 0:R1_J * F])
        nc.scalar.dma_start(t[:, R1_J:2 * N + 1, :], xn[r, R1_J * F:ROW])
        return t

    PREFETCH = 2
    pend = [issue_loads(0)]

    # Constants ride the SP ring behind group 0's loads (131 KiB, arrive
    # ~4 us — before the first transpose needs the identity) so the ACT
    # ring starts group 0's xn1 half at t=0.  All four 128x128 constants
    # share ONE tile and one DMA (and b|ones another): tile-pool releases
    # cost ~175 ns each in the end-of-kernel teardown, so fewer tile
    # instances directly shrink the fixed epilogue.
    cm = const.tile([128, 4 * 128], F16, tag="cm")
    nc.sync.dma_start(cm[:], cmat[:])
    ident = cm[:, 0:128]
    wS_t = cm[:, 128:256]
    w0_t = cm[:, 256:384]
    w1_t = cm[:, 384:512]
    bo = const.tile([1, D + 128], F16, tag="bo")
    nc.sync.dma_start(bo[:], bones[:])
    b_t = bo[:, 0:D]
    ones_t = bo[:, D:D + 128]

    for g in range(1, min(PREFETCH, ngroups)):
        pend.append(issue_loads(g))

    for g in range(ngroups):
        r = slice(g * GROUP, (g + 1) * GROUP)
        t = pend.pop(0)
        if g + PREFETCH < ngroups:
            pend.append(issue_loads(g + PREFETCH))

        # Fold each relation's 32 neighbour slices to 2 with four in-place
        # strided adds on the Vector engine (fp16 2x_1p), PER RELATION:
        # relation r's fold only depends on its own ring's DMA (subtile
        # deps), so folding xn0 overlaps xn1's load and each relation's
        # transposes overlap the other's fold.  The final 2->1 fold rides
        # the PE's PSUM accumulation below.
        #
        # One PSUM tile holds all three transposed operands side by side:
        # pacc[:, 0:128] = sum_n xn0 (as [f, bh]), [:, 128:256] = sum_n
        # xn1, [:, 256:384] = x_self.
        pp = ppool.tile([128, 5 * 128], F32, tag="pp")
        pacc = pp[:, 0:384]
        po = pp[:, 384:640]
        for rel in (0, 1):
            j0 = rel * R1_J
            for lv in (16, 8, 4, 2):
                nc.vector.tensor_add(t[:, j0:j0 + lv, :], t[:, j0:j0 + lv, :],
                                     t[:, j0 + lv:j0 + 2 * lv, :])
            c = slice(rel * 128, (rel + 1) * 128)
            nc.tensor.matmul(pacc[:, c], t[:, j0, :], ident[:],
                             start=True, stop=False)
            nc.tensor.matmul(pacc[:, c], t[:, j0 + 1, :], ident[:],
                             start=False, stop=True)
            if rel == 0:
                nc.tensor.matmul(pacc[:, 256:384], t[:, XS_J, :], ident[:],
                                 start=True, stop=True)

        # PSUM -> SBUF on the Scalar engine (keeps DVE free for folding),
        # casting to fp16 for the projection lhsT.
        wk = spool.tile([128, 5 * 128], F16, tag="wk")
        sacc = wk[:, 0:384]
        ob = wk[:, 384:640]
        nc.scalar.activation(sacc[:], pacc[:],
                             mybir.ActivationFunctionType.Copy)

        # Projection: out[bh, d]; bias broadcast via K=1 matmuls.
        nc.tensor.matmul(po[:, 0:HALF], sacc[:, 256:384], wS_t[:],
                         start=True, stop=False)
        nc.tensor.matmul(po[:, 0:HALF], ones_t[:], b_t[:, 0:HALF],
                         start=False, stop=True)
        nc.tensor.matmul(po[:, HALF:D], sacc[:, 0:128], w0_t[:],
                         start=True, stop=False)
        nc.tensor.matmul(po[:, HALF:D], sacc[:, 128:256], w1_t[:],
                         start=False, stop=False)
        nc.tensor.matmul(po[:, HALF:D], ones_t[:], b_t[:, HALF:D],
                         start=False, stop=True)

        # ReLU writes fp16 (the host upcasts to fp32) — halves store bytes.
        nc.scalar.activation(ob[:], po[:], mybir.ActivationFunctionType.Relu)
        nc.scalar.dma_start(out_d[r, :], ob[:])


def build_nc(ngroups=BH // GROUP):
    bh = ngroups * GROUP
    nc = bacc.Bacc("TRN2", target_bir_lowering=False, debug=False)
    xn = nc.dram_tensor("xn", [bh, ROW], F16, kind="ExternalInput")
    cmat = nc.dram_tensor("cmat", [128, 4 * 128], F16, kind="ExternalInput")
    bones = nc.dram_tensor("bones", [1, D + 128], F16, kind="ExternalInput")
    out = nc.dram_tensor("out", [bh, D], F16, kind="ExternalOutput")

    ins = [t.ap() for t in (xn, cmat, bones)]
    with tile.TileContext(nc) as tc:
        _tile_kernel(tc, [out.ap()], ins, ngroups)
    nc.compile()
    return nc


def make_in_maps(x_self, x_neigh_0, x_neigh_1, w_self, w_neigh_0, w_neigh_1, b):
    """Shard full inputs into per-core input maps (batch axis, 8 ways)."""
    x_self = np.asarray(x_self, dtype=np.float32).astype(np.float16)
    x_neigh_0 = np.asarray(x_neigh_0, dtype=np.float32).astype(np.float16)
    x_neigh_1 = np.asarray(x_neigh_1, dtype=np.float32).astype(np.float16)
    scale = np.float32(1.0 / (N * NR))
    w_s = np.asarray(w_self, dtype=np.float32).astype(np.float16)
    w0 = (np.asarray(w_neigh_0, dtype=np.float32) * scale).astype(np.float16)
    w1 = (np.asarray(w_neigh_1, dtype=np.float32) * scale).astype(np.float16)
    bvec = np.asarray(b, dtype=np.float32).astype(np.float16).reshape(1, D)
    ident = np.eye(128, dtype=np.float16)
    cmat = np.ascontiguousarray(np.hstack([ident, w_s, w0, w1]))
    bones = np.ascontiguousarray(
        np.hstack([bvec, np.ones((1, 128), dtype=np.float16)]))

    # Pack per row: xn[bh] = [xn0 | x_self | xn1]  (65 * 128 columns).
    xn_full = np.concatenate(
        [x_neigh_0.reshape(B * H, NF), x_self.reshape(B * H, F),
         x_neigh_1.reshape(B * H, NF)], axis=1)

    in_maps = []
    for c in range(NCORES):
        bs = slice(c * BSH * H, (c + 1) * BSH * H)
        in_maps.append({
            "xn": np.ascontiguousarray(xn_full[bs]),
            "cmat": cmat, "bones": bones,
        })
    return in_maps


_NC_CACHE = None


def kernel(x_self, x_neigh_0, x_neigh_1, w_self, w_neigh_0, w_neigh_1, b):
    global _NC_CACHE
    if _NC_CACHE is None:
        _NC_CACHE = build_nc()
    in_maps = make_in_maps(x_self, x_neigh_0, x_neigh_1,
                           w_self, w_neigh_0, w_neigh_1, b)
    res = bass_utils.run_bass_kernel_spmd(
        _NC_CACHE, in_maps, core_ids=list(range(NCORES)))
    out = np.concatenate([r["out"] for r in res.results], axis=0)
    return out.astype(np.float32).reshape(B, H, D)


# revision 26
# speedup vs baseline: 1.0280x; 1.0280x over previous
"""Trainium2 Bass kernel: MeanHinAggregator (GNN message passing).

Reference computation (per batch-head element bh):
    z_r  = mean_n(x_neigh_r[bh, n, :]) @ w_neigh_r          (r = 0, 1)
    out  = relu(concat(x_self[bh] @ w_self, (z0 + z1) / 2) + b)

Strategy (pure data parallel over 8 NeuronCores, batch axis sharded):
  * Per core: B_shard=128, H=10 -> 1280 rows, processed in 10 groups of 128.
  * The kernel is memory-bound (44 MB/core fp32; all 8 cores share one
    NeuronDevice's HBM, so the real constraint is the ~2.8 TB/s device
    aggregate).  All streamed tensors are downcast to fp16 on the host
    (rel-err gate is 2e-2; fp16 keeps us at ~4e-4): halves DMA bytes,
    doubles DVE fold throughput (2x_1p mode for 2-byte packed dtypes),
    and makes PE matmuls single-pass (1 cycle/row vs 4 for fp32).
  * Host packs [xn0 | x_self | xn1] row-wise into one [BH, 65*F] tensor;
    each group is one [128, 65, 128] fp16 tile loaded by TWO DMAs, one per
    HWDGE ring (SP gets xn0+xs, ACT gets xn1) -> balanced queues, one DMA
    + one tile instance fewer per group than a separate x_self stream, and
    each partition line is a contiguous >=8 KiB descriptor (full DMA rate).
  * xpool bufs=3 doubles as HBM pacing: a group's loads are gated on the
    group-3-back compute, capping per-core run-ahead so no core hogs the
    oversubscribed device HBM (bufs=2 serializes DMA with compute: +25 us;
    bufs=5 lets fast cores starve stragglers and raises the max).
  * The mean-over-neighbours reduction: four in-place strided adds on the
    Vector engine per relation fold its 32 slices to 2 (fp16 2x_1p,
    ~0.52 ns/elem); folding relation 0 only needs the SP-ring DMA, and its
    transposes overlap relation 1's fold.  NOTE: do NOT offload fold work
    to GPSIMD — DVE and GPSIMD share SBUF ports, so running them
    concurrently halves both (measured: 90us -> 100us).
  * Two accumulating transposing matmuls per relation (lhsT = data slice,
    rhs = identity -> out[f, bh] = data[bh, f], PSUM accumulation sums the
    last 2 slices) put the operands in the [f, bh] layout the projection
    needs as lhsT (the PE contracts over the partition axis).
  * Projection: out[bh, d] = sumT.T @ w.  The 1/(N*NR) mean scaling is
    folded into host-prescaled fp16 copies of w_neigh_*.  Bias is added
    with a K=1 matmul (lhsT = ones row, rhs = bias row) into PSUM.
    PSUM -> SBUF copy and final ReLU run on the Scalar engine (keeps DVE
    fold-only).  ReLU emits fp16; the host upcasts to fp32 (halves store
    bytes).  Output stores ride the ACT ring (SP's packed load is 3%
    bigger, so this balances the rings).

Measured on HW: 137.7 us (fp32 baseline) -> ~82-88 us best-of-10
(run-to-run device drift is +-5 us; all 8 cores share one NeuronDevice's
HBM and the kernel runs at the device-aggregate memory roofline).
"""

import numpy as np

import concourse.bacc as bacc
import concourse.bass as bass
import concourse.tile as tile
from concourse import bass_utils, mybir
from concourse._compat import with_exitstack

B, H, N, F = 1024, 10, 32, 128
HALF = 128
D = 2 * HALF
NR = 2
NCORES = 8
BSH = B // NCORES        # 128 batch rows per core
BH = BSH * H             # 1280 (bh rows per core)
GROUP = 128              # bh rows per group
NF = N * F               # 4096 (one relation's row width)
ROW = 2 * NF + F         # 8320 packed row: [xn0 | xs | xn1]
XS_J = N                 # j-index of the x_self slice in the packed tile
R1_J = N + 1             # j-index where relation 1's slices start
F32 = mybir.dt.float32
F16 = mybir.dt.float16


@with_exitstack
def _tile_kernel(ctx, tc, outs, ins, ngroups):
    nc = tc.nc
    xn, cmat, bones = ins
    (out_d,) = outs

    const = ctx.enter_context(tc.tile_pool(name="const", bufs=1))
    xpool = ctx.enter_context(tc.tile_pool(name="xp", bufs=3))
    spool = ctx.enter_context(tc.tile_pool(name="sp", bufs=3))
    ppool = ctx.enter_context(tc.tile_pool(name="ps", bufs=2, space="PSUM"))

    def issue_loads(g):
        """One packed tile per group, split across both HWDGE rings:
        SP carries xn0+xs (columns 0:4224), ACT carries xn1."""
        r = slice(g * GROUP, (g + 1) * GROUP)
        t = xpool.tile([128, 2 * N + 1, F], F16, tag="t")
        nc.sync.dma_start(t[:, 0:R1_J, :], xn[r, 0:R1_J * F])
        nc.scalar.dma_start(t[:, R1_J:2 * N + 1, :], xn[r, R1_J * F:ROW])
        return t

    PREFETCH = 2
    pend = [issue_loads(0)]

    # Constants ride the SP ring behind group 0's loads (131 KiB, arrive
    # ~4 us — before the first transpose needs the identity) so the ACT
    # ring starts group 0's xn1 half at t=0.  All four 128x128 constants
    # share ONE tile and one DMA (and b|ones another): tile-pool releases
    # cost ~175 ns each in the end-of-kernel teardown, so fewer tile
    # instances directly shrink the fixed epilogue.
    cm = const.tile([128, 4 * 128], F16, tag="cm")
    nc.sync.dma_start(cm[:], cmat[:])
    ident = cm[:, 0:128]
    wS_t = cm[:, 128:256]
    w0_t = cm[:, 256:384]
    w1_t = cm[:, 384:512]
    bo = const.tile([1, D + 128], F16, tag="bo")
    nc.sync.dma_start(bo[:], bones[:])
    b_t = bo[:, 0:D]
    ones_t = bo[:, D:D + 128]

    for g in range(1, min(PREFETCH, ngroups)):
        pend.append(issue_loads(g))

    for g in range(ngroups):
        r = slice(g * GROUP, (g + 1) * GROUP)
        t = pend.pop(0)
        if g + PREFETCH < ngroups:
            pend.append(issue_loads(g + PREFETCH))

        # Fold each relation's 32 neighbour slices to 2 with four in-place
        # strided adds on the Vector engine (fp16 2x_1p), PER RELATION:
        # relation r's fold only depends on its own ring's DMA (subtile
        # deps), so folding xn0 overlaps xn1's load and each relation's
        # transposes overlap the other's fold.  The final 2->1 fold rides
        # the PE's PSUM accumulation below.
        #
        # One PSUM tile holds all three transposed operands side by side:
        # pacc[:, 0:128] = sum_n xn0 (as [f, bh]), [:, 128:256] = sum_n
        # xn1, [:, 256:384] = x_self.
        pp = ppool.tile([128, 5 * 128], F32, tag="pp")
        pacc = pp[:, 0:384]
        po = pp[:, 384:640]
        for rel in (0, 1):
            j0 = rel * R1_J
            for lv in (16, 8, 4, 2):
                nc.vector.tensor_add(t[:, j0:j0 + lv, :], t[:, j0:j0 + lv, :],
                                     t[:, j0 + lv:j0 + 2 * lv, :])
            c = slice(rel * 128, (rel + 1) * 128)
            nc.tensor.matmul(pacc[:, c], t[:, j0, :], ident[:],
                             start=True, stop=False)
            nc.tensor.matmul(pacc[:, c], t[:, j0 + 1, :], ident[:],
                             start=False, stop=True)
            if rel == 0:
                nc.tensor.matmul(pacc[:, 256:384], t[:, XS_J, :], ident[:],
                                 start=True, stop=True)

        # PSUM -> SBUF on the Scalar engine (keeps DVE free for folding),
        # casting to fp16 for the projection lhsT.
        wk = spool.tile([128, 5 * 128], F16, tag="wk")
        sacc = wk[:, 0:384]
        ob = wk[:, 384:640]
        nc.scalar.activation(sacc[:], pacc[:],
                             mybir.ActivationFunctionType.Copy)

        # Projection: out[bh, d]; bias broadcast via K=1 matmuls.
        nc.tensor.matmul(po[:, 0:HALF], sacc[:, 256:384], wS_t[:],
                         start=True, stop=False)
        nc.tensor.matmul(po[:, 0:HALF], ones_t[:], b_t[:, 0:HALF],
                         start=False, stop=True)
        nc.tensor.matmul(po[:, HALF:D], sacc[:, 0:128], w0_t[:],
                         start=True, stop=False)
        nc.tensor.matmul(po[:, HALF:D], sacc[:, 128:256], w1_t[:],
                         start=False, stop=False)
        nc.tensor.matmul(po[:, HALF:D], ones_t[:], b_t[:, HALF:D],
                         start=False, stop=True)

        # ReLU writes fp16 (the host upcasts to fp32) — halves store bytes.
        nc.scalar.activation(ob[:], po[:], mybir.ActivationFunctionType.Relu)
        nc.scalar.dma_start(out_d[r, :], ob[:])


def build_nc(ngroups=BH // GROUP):
    bh = ngroups * GROUP
    nc = bacc.Bacc("TRN2", target_bir_lowering=False, debug=False)
    xn = nc.dram_tensor("xn", [bh, ROW], F16, kind="ExternalInput")
    cmat = nc.dram_tensor("cmat", [128, 4 * 128], F16, kind="ExternalInput")
    bones = nc.dram_tensor("bones", [1, D + 128], F16, kind="ExternalInput")
    out = nc.dram_tensor("out", [bh, D], F16, kind="ExternalOutput")

    ins = [t.ap() for t in (xn, cmat, bones)]
    with tile.TileContext(nc) as tc:
        _tile_kernel(tc, [out.ap()], ins, ngroups)
    nc.compile()
    return nc


def make_in_maps(x_self, x_neigh_0, x_neigh_1, w_self, w_neigh_0, w_neigh_1, b):
    """Shard full inputs into per-core input maps (batch axis, 8 ways)."""
    x_self = np.asarray(x_self, dtype=np.float32).astype(np.float16)
    x_neigh_0 = np.asarray(x_neigh_0, dtype=np.float32).astype(np.float16)
    x_neigh_1 = np.asarray(x_neigh_1, dtype=np.float32).astype(np.float16)
    scale = np.float32(1.0 / (N * NR))
    w_s = np.asarray(w_self, dtype=np.float32).astype(np.float16)
    w0 = (np.asarray(w_neigh_0, dtype=np.float32) * scale).astype(np.float16)
    w1 = (np.asarray(w_neigh_1, dtype=np.float32) * scale).astype(np.float16)
    bvec = np.asarray(b, dtype=np.float32).astype(np.float16).reshape(1, D)
    ident = np.eye(128, dtype=np.float16)
    cmat = np.ascontiguousarray(np.hstack([ident, w_s, w0, w1]))
    bones = np.ascontiguousarray(
        np.hstack([bvec, np.ones((1, 128), dtype=np.float16)]))

    # Pack per row: xn[bh] = [xn0 | x_self | xn1]  (65 * 128 columns).
    xn_full = np.concatenate(
        [x_neigh_0.reshape(B * H, NF), x_self.reshape(B * H, F),
         x_neigh_1.reshape(B * H, NF)], axis=1)

    in_maps = []
    for c in range(NCORES):
        bs = slice(c * BSH * H, (c + 1) * BSH * H)
        in_maps.append({
            "xn": np.ascontiguousarray(xn_full[bs]),
            "cmat": cmat, "bones": bones,
        })
    return in_maps


_NC_CACHE = None


def kernel(x_self, x_neigh_0, x_neigh_1, w_self, w_neigh_0, w_neigh_1, b):
    global _NC_CACHE
    if _NC_CACHE is None:
        _NC_CACHE = build_nc()
    in_maps = make_in_maps(x_self, x_neigh_0, x_neigh_1,
                           w_self, w_neigh_0, w_neigh_1, b)
    res = bass_utils.run_bass_kernel_spmd(
        _NC_CACHE, in_maps, core_ids=list(range(NCORES)))
    out = np.concatenate([r["out"] for r in res.results], axis=0)
    return out.astype(np.float32).reshape(B, H, D)
